# revision 9
# baseline (speedup 1.0000x reference)
"""Multi-head cross attention on 8 trn2 NeuronCores.

Problem: B=2, T=4096, EMB=512, H=8 heads (head dim 64), fp32 I/O.
  q = x1 @ Wq.T + bq ; k,v from x2 ; S = q k^T / sqrt(512) ;
  softmax over keys with -1e10 masking ; out = (A v) @ Wu.T + bu.

Sharding: core c handles batch b = c//4 and query rows
[1024*(c%4), 1024*(c%4+1)).  Each core computes K,V for its batch in
full (4-way duplication), its own Q chunk, attention, and out-proj.

The kernel is scalar-engine bound: exp() runs only on ACT at 1
elem/cycle/lane (dtype-independent), and the per-core score count is
8 heads x 1024 q x 4096 k = 33.5M elements ~ 255us.  The attention
loop is therefore structured so ACT never stalls:

  - scores for the head pair are computed into TWO per-head PSUM tiles
    sA/sB [128, QC] (2 banks each).  ACT(sA) runs while the next key
    tile's scores overwrite nothing (WAR via pool generation); the
    scores matmuls for key tile kk+1 execute on the PE during ACT(sB)
    of tile kk, so the scalar queue is always fed.
  - AV matmuls for tile kk-1 are emitted after the scores of kk, so
    the PE queue is [scores(kk), AV(kk-1), scores(kk+1), ...] and
    never waits on the exp/mask chain.
  - PSUM budget: sA + sB (4 banks) + avA + avB accumulators (4 banks)
    = all 8 banks; projections run in a serial prologue with their own
    (then freed) PSUM pool.
  - mask tiles stream through a 4-deep SBUF ring, one DMA per
    (pr, kk); the P = exp(S)*mask multiply runs on DVE at 2x fp16
    rate; softmax normalization is deferred via a ones-column in V
    (row 64 of the AV accumulator = denominator), with the reciprocal
    broadcast across partitions on the (otherwise idle) GPSIMD engine.
"""
import math
import os
from contextlib import ExitStack

import numpy as np

import concourse.bass as bass
import concourse.bacc as bacc
import concourse.tile as tile
import concourse.mybir as mybir
from concourse.bass_utils import run_bass_kernel_spmd

F16 = mybir.dt.float16
F32 = mybir.dt.float32
EXP = mybir.ActivationFunctionType.Exp

EMB, H, D, CT = 512, 8, 64, 4  # emb, heads, head dim, emb/128

FULL_CFG = dict(T=4096, QC=1024)  # keys per batch, query rows per core
MINI_CFG = dict(T=512, QC=256)


def attention_body(ctx, tc, io, cfg):
    nc = tc.nc
    T, QC = cfg["T"], cfg["QC"]
    KT = T // 128            # key tiles
    QW = min(512, QC)        # matmul moving width (PSUM bank limit)
    NB = QC // QW            # query blocks
    scale = 1.0 / math.sqrt(EMB)

    pw = ctx.enter_context(tc.tile_pool(name="w", bufs=1))
    pk = ctx.enter_context(tc.tile_pool(name="kt", bufs=1))
    pv = ctx.enter_context(tc.tile_pool(name="v", bufs=1))
    pq = ctx.enter_context(tc.tile_pool(name="qt", bufs=1))
    py = ctx.enter_context(tc.tile_pool(name="y", bufs=1))

    # persistent weights / biases / constants
    wq = [pw.tile([128, EMB], F16, tag=f"wq{i}", name=f"wq{i}") for i in range(CT)]
    wk = [pw.tile([128, EMB], F16, tag=f"wk{i}", name=f"wk{i}") for i in range(CT)]
    wv = [pw.tile([128, EMB], F16, tag=f"wv{i}", name=f"wv{i}") for i in range(CT)]
    wu = [pw.tile([128, EMB], F16, tag=f"wu{i}", name=f"wu{i}") for i in range(CT)]
    for i in range(CT):
        nc.sync.dma_start(wq[i][:], io["wqT"][bass.ts(i, 128), :])
        nc.sync.dma_start(wk[i][:], io["wkT"][bass.ts(i, 128), :])
        nc.sync.dma_start(wv[i][:], io["wvT"][bass.ts(i, 128), :])
        nc.sync.dma_start(wu[i][:], io["wuT"][bass.ts(i, 128), :])
    bqr = pw.tile([128, CT], F32, tag="bqr", name="bqr")
    bkr = pw.tile([128, CT], F32, tag="bkr", name="bkr")
    bvb = pw.tile([128, EMB], F32, tag="bvb", name="bvb")
    bub = pw.tile([128, EMB], F32, tag="bub", name="bub")
    nc.sync.dma_start(bqr[:], io["bqr"][:, :])
    nc.sync.dma_start(bkr[:], io["bkr"][:, :])
    nc.sync.dma_start(bvb[:], io["bvb"][:, :])
    nc.sync.dma_start(bub[:], io["bub"][:, :])

    # preload the exp table while DMAs run (one-time ~2.7us)
    warm = pw.tile([1, 1], F32, tag="warm", name="warm")
    nc.vector.memset(warm[:], 0.0)
    nc.scalar.activation(warm[:], warm[:], EXP)

    ones = pw.tile([128, 1], F16, tag="ones", name="ones")
    nc.vector.memset(ones[:], 1.0)
    ones1 = pw.tile([1, D], F16, tag="ones1", name="ones1")
    nc.vector.memset(ones1[:], 1.0)

    # persistent K^T [emb, T], V [key, head, 64], Q^T [emb, QC]
    kt = [pk.tile([128, T], F16, tag=f"kt{i}", name=f"kt{i}") for i in range(CT)]
    v = pv.tile([128, KT, H, 64], F16, tag="v", name="v")
    qt = [pq.tile([128, QC], F16, tag=f"qt{i}", name=f"qt{i}") for i in range(CT)]
    yts = [py.tile([128, QC], F16, tag=f"yt{e}", name=f"yt{e}") for e in range(CT)]

    # ---- prologue: projections (own PSUM pool, freed before attention) ----
    with tc.tile_pool(name="x", bufs=1) as px, \
         tc.tile_pool(name="pp", bufs=1, space="PSUM") as pp:
        x2t = [px.tile([128, T], F16, tag=f"x2t{i}", name=f"x2t{i}") for i in range(CT)]
        x1t = [px.tile([128, QC], F16, tag=f"x1t{i}", name=f"x1t{i}") for i in range(CT)]
        for i in range(CT):
            nc.sync.dma_start(x1t[i][:], io["x1T"][bass.ts(i, 128), :])
            for hf in range(2):
                nc.sync.dma_start(x2t[i][:, bass.ts(hf, T // 2)],
                                  io["x2T"][bass.ts(i, 128), bass.ts(hf, T // 2)])

        # Q^T[e,q] = sum_c WqT[c,e] * x1T[c,q]  (+ bq per-partition)
        for e in range(CT):
            for t in range(NB):
                ps = pp.tile([128, QW], F32, tag=f"ps{t}", name="ps2")
                for c in range(CT):
                    nc.tensor.matmul(ps[:], wq[c][:, bass.ts(e, 128)],
                                     x1t[c][:, bass.ts(t, QW)],
                                     start=(c == 0), stop=(c == CT - 1))
                nc.vector.tensor_scalar_add(qt[e][:, bass.ts(t, QW)], ps[:],
                                            bqr[:, e:e + 1])
        # K^T[e,t] = sum_c WkT[c,e] * x2T[c,t] (+ bk); V[t,e] interleaved.
        # Stationary (wk) reused across 8 t-chunks per LDWEIGHTS.
        NT8 = min(8, T // 512)
        for e in range(CT):
            for tb in range(T // 512 // NT8):
                pss = [pp.tile([128, 512], F32, tag=f"ps{j}", name=f"ps{j}")
                       for j in range(NT8)]
                for c in range(CT):
                    for j in range(NT8):
                        nc.tensor.matmul(pss[j][:], wk[c][:, bass.ts(e, 128)],
                                         x2t[c][:, bass.ts(tb * NT8 + j, 512)],
                                         start=(c == 0), stop=(c == CT - 1))
                for j in range(NT8):
                    nc.vector.tensor_scalar_add(
                        kt[e][:, bass.ts(tb * NT8 + j, 512)], pss[j][:],
                        bkr[:, e:e + 1])
            if e > 0:
                continue
            # V right after K^T e-tile 0
            for t in range(KT):
                ps = pp.tile([128, EMB], F32, tag="ps0", name="psv")
                for c in range(CT):
                    nc.tensor.matmul(ps[:], x2t[c][:, bass.ts(t, 128)], wv[c][:],
                                     start=(c == 0), stop=(c == CT - 1))
                nc.vector.tensor_add(
                    v[:, t, :, :],
                    ps[:].rearrange("p (h d) -> p h d", h=H),
                    bvb[:].rearrange("p (h d) -> p h d", h=H))

    # ---- attention: ACT-saturated pipeline ----
    # PSUM: sA+sB (4 banks) + av merged col-tiled (2) + r/bc rotation (2) = 8.
    MU = mybir.AluOpType.mult
    AD = mybir.AluOpType.add
    with tc.tile_pool(name="ps_s", bufs=1, space="PSUM") as ps_s, \
         tc.tile_pool(name="ps_av", bufs=1, space="PSUM") as ps_av, \
         tc.tile_pool(name="ps_r", bufs=1, space="PSUM") as ps_r, \
         tc.tile_pool(name="pe", bufs=2) as pe, \
         tc.tile_pool(name="ppt", bufs=2) as ppt, \
         tc.tile_pool(name="pm", bufs=4) as pm, \
         tc.tile_pool(name="pacc", bufs=1) as pacc, \
         tc.tile_pool(name="pn", bufs=2) as pn, \
         tc.tile_pool(name="po", bufs=2) as po:
        for pr in range(CT):  # head pair (= e-tile of Q/K)
            # both heads' AV accumulators col-tiled into one 2-bank tile:
            # rows 0-63 = head A (array cols 0-63), 64-127 = head B.
            av = ps_av.tile([128, QC], F32, tag="av", name="av")
            acc = [pacc.tile([128, QC], F16, tag=f"acc{hh}", name=f"acc{hh}")
                   for hh in range(2)]

            def emit_av(kk, pts):
                for hh in range(2):
                    for cb in range(NB):
                        nc.tensor.matmul(
                            av[bass.ds(64 * hh, 64), bass.ts(cb, QW)],
                            v[:, kk, 2 * pr + hh, :],
                            pts[hh][:, bass.ts(cb, QW)],
                            start=(kk == 0), stop=(kk == KT - 1),
                            tile_position=(0, 64 * hh))

            prev = None
            for kk in range(KT):
                mkt = pm.tile([128, QC], F16, tag="mk", name="mk")
                nc.sync.dma_start(mkt[:], io["maskT"][bass.ts(kk, 128), :])
                ss = [ps_s.tile([128, QC], F32, tag=f"s{hh}", name=f"s{hh}")
                      for hh in range(2)]
                # scores: row-packed head pair, separate PSUM tiles
                for cb in range(NB):
                    for hh in range(2):
                        nc.tensor.matmul(
                            ss[hh][:, bass.ts(cb, QW)],
                            kt[pr][bass.ds(64 * hh, 64), bass.ts(kk, 128)],
                            qt[pr][bass.ds(64 * hh, 64), bass.ts(cb, QW)],
                            start=True, stop=True,
                            tile_position=(64 * hh, 0))
                pts = []
                for hh in range(2):
                    e16 = pe.tile([128, QC], F16, tag=f"e{hh}", name=f"e{hh}")
                    nc.scalar.activation(e16[:], ss[hh][:], EXP, scale=scale)
                    pt = ppt.tile([128, QC], F16, tag=f"p{hh}", name=f"p{hh}")
                    # P = exp(S) * mask, then acc += P (both 4x fp16 DVE)
                    nc.vector.scalar_tensor_tensor(pt[:], e16[:], 1.0, mkt[:],
                                                   MU, MU)
                    if kk == 0:
                        nc.vector.tensor_copy(acc[hh][:], pt[:])
                    else:
                        nc.vector.scalar_tensor_tensor(acc[hh][:], pt[:], 1.0,
                                                       acc[hh][:], MU, AD)
                    pts.append(pt)
                if prev is not None:
                    emit_av(*prev)
                prev = (kk, pts)
            emit_av(*prev)

            # normalize: Y^T_h / r_h; r_h = ones^T acc_h via K=1 matmuls,
            # reciprocal broadcast to 64 rows via col-tiled K=1 matmuls.
            rrs = []
            for hh in range(2):
                r_ps = ps_r.tile([1, QC], F32, tag="r", name="r")
                for cb in range(NB):
                    nc.tensor.matmul(r_ps[:, bass.ts(cb, QW)], ones[:],
                                     acc[hh][:, bass.ts(cb, QW)],
                                     start=True, stop=True)
                rr32 = pn.tile([1, QC], F32, tag=f"rr32{hh}", name="rr32")
                nc.vector.reciprocal_approx_fast(rr32[:], r_ps[:])
                rr = pn.tile([1, QC], F16, tag=f"rr{hh}", name="rr")
                with nc.allow_low_precision(reason="fp16 recip copy ok"):
                    nc.vector.tensor_copy(rr[:], rr32[:])
                rrs.append(rr)
            bc = ps_r.tile([128, QC], F32, tag="r", name="bc")
            for hh in range(2):
                for cb in range(NB):
                    nc.tensor.matmul(bc[bass.ds(64 * hh, 64), bass.ts(cb, QW)],
                                     ones1[:], rrs[hh][:, bass.ts(cb, QW)],
                                     start=True, stop=True,
                                     tile_position=(0, 64 * hh))
            bc16 = pn.tile([128, QC], F16, tag="bc16", name="bc16")
            with nc.allow_low_precision(reason="fp16 recip bc ok"):
                nc.vector.tensor_copy(bc16[:], bc[:])
            with nc.allow_low_precision(reason="fp16 y norm ok"):
                nc.vector.tensor_mul(yts[pr][:], av[:], bc16[:])

        # out[q, :] = sum_e Y^T[e, q] * WuT[e, :] + bu
        for qi in range(QC // 128):
            pso = ps_s.tile([128, max(QC, EMB)], F32, tag="s0", name="pso")
            for e in range(CT):
                nc.tensor.matmul(pso[:, 0:EMB], yts[e][:, bass.ts(qi, 128)],
                                 wu[e][:], start=(e == 0), stop=(e == CT - 1))
            osb = po.tile([128, EMB], F32, tag="o", name="osb")
            nc.vector.tensor_add(osb[:], pso[:, 0:EMB], bub[:])
            nc.sync.dma_start(io["out"][bass.ts(qi, 128), :], osb[:])


def build(cfg, num_devices=8):
    T, QC = cfg["T"], cfg["QC"]
    nc = bacc.Bacc("TRN2", target_bir_lowering=False, debug=False,
                   num_devices=num_devices)
    io = {
        "x1T": nc.dram_tensor("x1T", [EMB, QC], F16, kind="ExternalInput").ap(),
        "x2T": nc.dram_tensor("x2T", [EMB, T], F16, kind="ExternalInput").ap(),
        "maskT": nc.dram_tensor("maskT", [T, QC], F16, kind="ExternalInput").ap(),
        "wqT": nc.dram_tensor("wqT", [EMB, EMB], F16, kind="ExternalInput").ap(),
        "wkT": nc.dram_tensor("wkT", [EMB, EMB], F16, kind="ExternalInput").ap(),
        "wvT": nc.dram_tensor("wvT", [EMB, EMB], F16, kind="ExternalInput").ap(),
        "wuT": nc.dram_tensor("wuT", [EMB, EMB], F16, kind="ExternalInput").ap(),
        "bqr": nc.dram_tensor("bqr", [128, CT], F32, kind="ExternalInput").ap(),
        "bkr": nc.dram_tensor("bkr", [128, CT], F32, kind="ExternalInput").ap(),
        "bvb": nc.dram_tensor("bvb", [128, EMB], F32, kind="ExternalInput").ap(),
        "bub": nc.dram_tensor("bub", [128, EMB], F32, kind="ExternalInput").ap(),
        "out": nc.dram_tensor("out", [QC, EMB], F32, kind="ExternalOutput").ap(),
    }
    with tile.TileContext(nc) as tc:
        with ExitStack() as ctx:
            attention_body(ctx, tc, io, cfg)
    nc.compile()
    return nc


def host_prep(x1, x2, mask, Wq, bq, Wk, bk, Wv, bv, Wu, bu, cfg):
    """Build the 8 per-core input maps from full inputs."""
    T, QC = cfg["T"], cfg["QC"]
    shared = {
        "wqT": np.ascontiguousarray(Wq.T).astype(np.float16),
        "wkT": np.ascontiguousarray(Wk.T).astype(np.float16),
        "wvT": np.ascontiguousarray(Wv.T).astype(np.float16),
        "wuT": np.ascontiguousarray(Wu.T).astype(np.float16),
        "bqr": np.ascontiguousarray(bq.reshape(CT, 128).T).astype(np.float32),
        "bkr": np.ascontiguousarray(bk.reshape(CT, 128).T).astype(np.float32),
        "bvb": np.ascontiguousarray(np.broadcast_to(bv, (128, EMB))).astype(np.float32),
        "bub": np.ascontiguousarray(np.broadcast_to(bu, (128, EMB))).astype(np.float32),
    }
    x2T = [x2[b].T.astype(np.float16) for b in range(x1.shape[0])]
    in_maps = []
    n_cores = (x1.shape[0] * x1.shape[1]) // QC
    per_b = x1.shape[1] // QC
    for c in range(n_cores):
        b, q0 = c // per_b, (c % per_b) * QC
        in_maps.append(dict(
            shared,
            x1T=x1[b, q0:q0 + QC, :].T.astype(np.float16),
            x2T=x2T[b],
            maskT=mask[b, q0:q0 + QC, :].T.astype(np.float16),
        ))
    return in_maps


_NC_CACHE = {}


def kernel(x1, x2, mask, Wq, bq, Wk, bk, Wv, bv, Wu, bu):
    cfg = FULL_CFG
    B, TQ, _ = x1.shape
    in_maps = host_prep(np.asarray(x1, np.float32), np.asarray(x2, np.float32),
                        np.asarray(mask), np.asarray(Wq, np.float32),
                        np.asarray(bq, np.float32), np.asarray(Wk, np.float32),
                        np.asarray(bk, np.float32), np.asarray(Wv, np.float32),
                        np.asarray(bv, np.float32), np.asarray(Wu, np.float32),
                        np.asarray(bu, np.float32), cfg)
    key = (cfg["T"], cfg["QC"])
    if key not in _NC_CACHE:
        _NC_CACHE[key] = build(cfg)
    nc = _NC_CACHE[key]
    res = run_bass_kernel_spmd(nc, in_maps, core_ids=list(range(8)),
                               trace=bool(os.environ.get("KERNEL_TRACE")))
    if os.environ.get("KERNEL_TRACE"):
        kernel.last_exec_ns = res.exec_time_ns
        kernel.last_results = res
    out = np.empty((B, TQ, EMB), np.float32)
    per_b = TQ // cfg["QC"]
    for c in range(8):
        b, q0 = c // per_b, (c % per_b) * cfg["QC"]
        out[b, q0:q0 + cfg["QC"], :] = res.results[c]["out"]
    return out


# revision 16
# speedup vs baseline: 1.5164x; 1.5164x over previous
"""Multi-head cross attention on 8 trn2 NeuronCores.

Problem: B=2, T=4096, EMB=512, H=8 heads (head dim 64), fp32 I/O.
  q = x1 @ Wq.T + bq ; k,v from x2 ; S = q k^T / sqrt(512) ;
  softmax over keys with -1e10 masking ; out = (A v) @ Wu.T + bu.

Sharding: core c handles batch b = c//4 and query rows
[1024*(c%4), 1024*(c%4+1)).  Each core computes K,V for its batch in
full (4-way duplication), its own Q chunk, attention, and out-proj.

The kernel is scalar-engine bound: exp() runs only on ACT at 1
elem/cycle/lane (dtype-independent), and the per-core score count is
8 heads x 1024 q x 4096 k = 33.5M elements ~ 255us.  The attention
loop is therefore structured so ACT never stalls:

  - scores for the head pair are computed into TWO per-head PSUM tiles
    sA/sB [128, QC] (2 banks each).  ACT(sA) runs while the next key
    tile's scores overwrite nothing (WAR via pool generation); the
    scores matmuls for key tile kk+1 execute on the PE during ACT(sB)
    of tile kk, so the scalar queue is always fed.
  - AV matmuls for tile kk-1 are emitted after the scores of kk, so
    the PE queue is [scores(kk), AV(kk-1), scores(kk+1), ...] and
    never waits on the exp/mask chain.
  - PSUM budget: sA + sB (4 banks) + avA + avB accumulators (4 banks)
    = all 8 banks; projections run in a serial prologue with their own
    (then freed) PSUM pool.
  - mask tiles stream through a 4-deep SBUF ring, one DMA per
    (pr, kk); the P = exp(S)*mask multiply runs on DVE at 2x fp16
    rate; softmax normalization is deferred via a ones-column in V
    (row 64 of the AV accumulator = denominator), with the reciprocal
    broadcast across partitions on the (otherwise idle) GPSIMD engine.
"""
import math
import os
from contextlib import ExitStack

import numpy as np

import concourse.bass as bass
import concourse.bacc as bacc
import concourse.tile as tile
import concourse.mybir as mybir
from concourse.bass_utils import run_bass_kernel_spmd

F16 = mybir.dt.float16
F32 = mybir.dt.float32
EXP = mybir.ActivationFunctionType.Exp

EMB, H, D, CT = 512, 8, 64, 4  # emb, heads, head dim, emb/128

FULL_CFG = dict(T=4096, QC=1024)  # keys per batch, query rows per core
MINI_CFG = dict(T=512, QC=256)


def attention_body(ctx, tc, io, cfg):
    nc = tc.nc
    T, QC = cfg["T"], cfg["QC"]
    KT = T // 128            # key tiles
    QW = min(512, QC)        # matmul moving width (PSUM bank limit)
    NB = QC // QW            # query blocks
    scale = 1.0 / math.sqrt(EMB)

    pw = ctx.enter_context(tc.tile_pool(name="w", bufs=1))
    pk = ctx.enter_context(tc.tile_pool(name="kt", bufs=1))
    pv = ctx.enter_context(tc.tile_pool(name="v", bufs=1))
    pq = ctx.enter_context(tc.tile_pool(name="qt", bufs=1))
    py = ctx.enter_context(tc.tile_pool(name="y", bufs=1))

    # persistent weights / biases / constants
    wq = [pw.tile([128, EMB], F16, tag=f"wq{i}", name=f"wq{i}") for i in range(CT)]
    wk = [pw.tile([128, EMB], F16, tag=f"wk{i}", name=f"wk{i}") for i in range(CT)]
    wv = [pw.tile([128, EMB], F16, tag=f"wv{i}", name=f"wv{i}") for i in range(CT)]
    wu = [pw.tile([128, EMB], F16, tag=f"wu{i}", name=f"wu{i}") for i in range(CT)]
    for i in range(CT):
        nc.sync.dma_start(wq[i][:], io["wqT"][bass.ts(i, 128), :])
        nc.sync.dma_start(wk[i][:], io["wkT"][bass.ts(i, 128), :])
        nc.sync.dma_start(wv[i][:], io["wvT"][bass.ts(i, 128), :])
        nc.sync.dma_start(wu[i][:], io["wuT"][bass.ts(i, 128), :])
    bqr = pw.tile([128, CT], F32, tag="bqr", name="bqr")
    bkr = pw.tile([128, CT], F32, tag="bkr", name="bkr")
    bvb = pw.tile([128, EMB], F32, tag="bvb", name="bvb")
    bub = pw.tile([128, EMB], F32, tag="bub", name="bub")
    nc.sync.dma_start(bqr[:], io["bqr"][:, :])
    nc.sync.dma_start(bkr[:], io["bkr"][:, :])
    nc.sync.dma_start(bvb[:], io["bvb"][:, :])
    nc.sync.dma_start(bub[:], io["bub"][:, :])

    # preload the exp table while DMAs run (one-time ~2.7us)
    warm = pw.tile([1, 1], F32, tag="warm", name="warm")
    nc.vector.memset(warm[:], 0.0)
    nc.scalar.activation(warm[:], warm[:], EXP)

    # [1|0] and [0|1] two-column stationaries: both heads' r accumulate in
    # one [2, QC] PSUM region (each head's zero column adds 0 to the other row)
    onz = pw.tile([128, 2, 2], F16, tag="onz", name="onz")
    nc.vector.memset(onz[:], 0.0)
    nc.vector.memset(onz[:, 0, 0:1], 1.0)
    nc.vector.memset(onz[:, 1, 1:2], 1.0)
    # row-select stationaries for the K=2 reciprocal-broadcast matmuls
    sel2 = pw.tile([2, 2 * D], F16, tag="sel2", name="sel2")
    nc.sync.dma_start(sel2[:], io["sel2"][:, :])

    # persistent K^T [emb, T], V [key, head, 64], Q^T [emb, QC]
    kt = [pk.tile([128, T], F16, tag=f"kt{i}", name=f"kt{i}") for i in range(CT)]
    v = pv.tile([128, KT, H, 64], F16, tag="v", name="v")
    qt = [pq.tile([128, QC], F16, tag=f"qt{i}", name=f"qt{i}") for i in range(CT)]
    yts = [py.tile([128, QC], F16, tag=f"yt{e}", name=f"yt{e}") for e in range(CT)]

    # ---- prologue: projections (own PSUM pool, freed before attention) ----
    with tc.tile_pool(name="x", bufs=1) as px, \
         tc.tile_pool(name="pp", bufs=1, space="PSUM") as pp:
        x2t = [px.tile([128, T], F16, tag=f"x2t{i}", name=f"x2t{i}") for i in range(CT)]
        x1t = [px.tile([128, QC], F16, tag=f"x1t{i}", name=f"x1t{i}") for i in range(CT)]
        for i in range(CT):
            nc.sync.dma_start(x1t[i][:], io["x1T"][bass.ts(i, 128), :])
            for hf in range(2):
                nc.sync.dma_start(x2t[i][:, bass.ts(hf, T // 2)],
                                  io["x2T"][bass.ts(i, 128), bass.ts(hf, T // 2)])

        # Q^T[e,q] = sum_c WqT[c,e] * x1T[c,q]  (+ bq per-partition)
        for e in range(CT):
            for t in range(NB):
                ps = pp.tile([128, QW], F32, tag=f"ps{t}", name="ps2")
                for c in range(CT):
                    nc.tensor.matmul(ps[:], wq[c][:, bass.ts(e, 128)],
                                     x1t[c][:, bass.ts(t, QW)],
                                     start=(c == 0), stop=(c == CT - 1))
                nc.vector.tensor_scalar_add(qt[e][:, bass.ts(t, QW)], ps[:],
                                            bqr[:, e:e + 1])
        # K^T[e,t] = sum_c WkT[c,e] * x2T[c,t] (+ bk); V[t,e] interleaved.
        # Stationary (wk) reused across 8 t-chunks per LDWEIGHTS.
        NT8 = min(8, T // 512)
        for e in range(CT):
            for tb in range(T // 512 // NT8):
                pss = [pp.tile([128, 512], F32, tag=f"ps{j}", name=f"ps{j}")
                       for j in range(NT8)]
                for c in range(CT):
                    for j in range(NT8):
                        nc.tensor.matmul(pss[j][:], wk[c][:, bass.ts(e, 128)],
                                         x2t[c][:, bass.ts(tb * NT8 + j, 512)],
                                         start=(c == 0), stop=(c == CT - 1))
                for j in range(NT8):
                    nc.vector.tensor_scalar_add(
                        kt[e][:, bass.ts(tb * NT8 + j, 512)], pss[j][:],
                        bkr[:, e:e + 1])
            if e > 0:
                continue
            # V right after K^T e-tile 0
            for t in range(KT):
                ps = pp.tile([128, EMB], F32, tag="ps0", name="psv")
                for c in range(CT):
                    nc.tensor.matmul(ps[:], x2t[c][:, bass.ts(t, 128)], wv[c][:],
                                     start=(c == 0), stop=(c == CT - 1))
                nc.vector.tensor_add(
                    v[:, t, :, :],
                    ps[:].rearrange("p (h d) -> p h d", h=H),
                    bvb[:].rearrange("p (h d) -> p h d", h=H))

    # ---- attention: ACT-saturated pipeline ----
    # PSUM: sA+sB (4 banks) + av merged col-tiled (2) + r/bc rotation (2) = 8.
    with tc.tile_pool(name="ps_s", bufs=1, space="PSUM") as ps_s, \
         tc.tile_pool(name="ps_av", bufs=1, space="PSUM") as ps_av, \
         tc.tile_pool(name="ps_r", bufs=1, space="PSUM") as ps_r, \
         tc.tile_pool(name="pe", bufs=2) as pe, \
         tc.tile_pool(name="ppt", bufs=2) as ppt, \
         tc.tile_pool(name="pm", bufs=4) as pm, \
         tc.tile_pool(name="pn", bufs=2) as pn, \
         tc.tile_pool(name="po", bufs=2) as po:
        for pr in range(CT):  # head pair (= e-tile of Q/K)
            # both heads' AV accumulators col-tiled into one 2-bank tile:
            # rows 0-63 = head A (array cols 0-63), 64-127 = head B.
            av = ps_av.tile([128, QC], F32, tag="av", name="av")
            # r accumulator: row 0 = head A, row 1 = head B (onz zero-cols)
            r_ps = ps_r.tile([2, QC], F32, tag="r", name="r")

            def emit_av(kk, pts):
                # interleave col groups so the A/B matmuls run concurrently
                for cb in range(NB):
                    for hh in range(2):
                        nc.tensor.matmul(
                            av[bass.ds(64 * hh, 64), bass.ts(cb, QW)],
                            v[:, kk, 2 * pr + hh, :],
                            pts[hh][:, bass.ts(cb, QW)],
                            start=(kk == 0), stop=(kk == KT - 1),
                            tile_position=(0, 64 * hh))
                for cb in range(NB):
                    for hh in range(2):
                        nc.tensor.matmul(
                            r_ps[:, bass.ts(cb, QW)], onz[:, hh, :],
                            pts[hh][:, bass.ts(cb, QW)],
                            start=(kk == 0 and hh == 0), stop=(kk == KT - 1))

            prev = None
            for kk in range(KT):
                mkt = pm.tile([128, QC], F16, tag="mk", name="mk")
                nc.sync.dma_start(mkt[:], io["maskT"][bass.ts(kk, 128), :])
                ss = [ps_s.tile([128, QC], F32, tag=f"s{hh}", name=f"s{hh}")
                      for hh in range(2)]
                # scores: row-packed head pair, separate PSUM tiles
                for cb in range(NB):
                    for hh in range(2):
                        nc.tensor.matmul(
                            ss[hh][:, bass.ts(cb, QW)],
                            kt[pr][bass.ds(64 * hh, 64), bass.ts(kk, 128)],
                            qt[pr][bass.ds(64 * hh, 64), bass.ts(cb, QW)],
                            start=True, stop=True,
                            tile_position=(64 * hh, 0))
                pts = []
                for hh in range(2):
                    e16 = pe.tile([128, QC], F16, tag=f"e{hh}", name=f"e{hh}")
                    nc.scalar.activation(e16[:], ss[hh][:], EXP, scale=scale)
                    pt = ppt.tile([128, QC], F16, tag=f"p{hh}", name=f"p{hh}")
                    nc.vector.tensor_mul(pt[:], e16[:], mkt[:])
                    pts.append(pt)
                if prev is not None:
                    emit_av(*prev)
                prev = (kk, pts)
            emit_av(*prev)

            # normalize: Y^T_h / r_h via reciprocal + col-tiled K=2 broadcast
            rr32 = pn.tile([2, QC], F32, tag="rr32", name="rr32")
            nc.vector.reciprocal_approx_fast(rr32[:], r_ps[:])
            rr = pn.tile([2, QC], F16, tag="rr", name="rr")
            with nc.allow_low_precision(reason="fp16 recip copy ok"):
                nc.vector.tensor_copy(rr[:], rr32[:])
            bc = ps_r.tile([128, QC], F32, tag="r", name="bc")
            for hh in range(2):
                for cb in range(NB):
                    nc.tensor.matmul(bc[bass.ds(64 * hh, 64), bass.ts(cb, QW)],
                                     sel2[:, bass.ts(hh, D)],
                                     rr[:, bass.ts(cb, QW)],
                                     start=True, stop=True,
                                     tile_position=(0, 64 * hh))
            bc16 = pn.tile([128, QC], F16, tag="bc16", name="bc16")
            with nc.allow_low_precision(reason="fp16 recip bc ok"):
                nc.vector.tensor_copy(bc16[:], bc[:])
            with nc.allow_low_precision(reason="fp16 y norm ok"):
                nc.vector.tensor_mul(yts[pr][:], av[:], bc16[:])

        # out[q, :] = sum_e Y^T[e, q] * WuT[e, :] + bu
        for qi in range(QC // 128):
            pso = ps_s.tile([128, max(QC, EMB)], F32, tag="s0", name="pso")
            for e in range(CT):
                nc.tensor.matmul(pso[:, 0:EMB], yts[e][:, bass.ts(qi, 128)],
                                 wu[e][:], start=(e == 0), stop=(e == CT - 1))
            osb = po.tile([128, EMB], F32, tag="o", name="osb")
            nc.vector.tensor_add(osb[:], pso[:, 0:EMB], bub[:])
            nc.sync.dma_start(io["out"][bass.ts(qi, 128), :], osb[:])


def build(cfg, num_devices=8):
    T, QC = cfg["T"], cfg["QC"]
    nc = bacc.Bacc("TRN2", target_bir_lowering=False, debug=False,
                   num_devices=num_devices)
    io = {
        "x1T": nc.dram_tensor("x1T", [EMB, QC], F16, kind="ExternalInput").ap(),
        "x2T": nc.dram_tensor("x2T", [EMB, T], F16, kind="ExternalInput").ap(),
        "maskT": nc.dram_tensor("maskT", [T, QC], F16, kind="ExternalInput").ap(),
        "wqT": nc.dram_tensor("wqT", [EMB, EMB], F16, kind="ExternalInput").ap(),
        "wkT": nc.dram_tensor("wkT", [EMB, EMB], F16, kind="ExternalInput").ap(),
        "wvT": nc.dram_tensor("wvT", [EMB, EMB], F16, kind="ExternalInput").ap(),
        "wuT": nc.dram_tensor("wuT", [EMB, EMB], F16, kind="ExternalInput").ap(),
        "bqr": nc.dram_tensor("bqr", [128, CT], F32, kind="ExternalInput").ap(),
        "bkr": nc.dram_tensor("bkr", [128, CT], F32, kind="ExternalInput").ap(),
        "bvb": nc.dram_tensor("bvb", [128, EMB], F32, kind="ExternalInput").ap(),
        "bub": nc.dram_tensor("bub", [128, EMB], F32, kind="ExternalInput").ap(),
        "sel2": nc.dram_tensor("sel2", [2, 2 * D], F16, kind="ExternalInput").ap(),
        "out": nc.dram_tensor("out", [QC, EMB], F32, kind="ExternalOutput").ap(),
    }
    with tile.TileContext(nc) as tc:
        with ExitStack() as ctx:
            attention_body(ctx, tc, io, cfg)
    nc.compile()
    return nc


def host_prep(x1, x2, mask, Wq, bq, Wk, bk, Wv, bv, Wu, bu, cfg):
    """Build the 8 per-core input maps from full inputs."""
    T, QC = cfg["T"], cfg["QC"]
    shared = {
        "wqT": np.ascontiguousarray(Wq.T).astype(np.float16),
        "wkT": np.ascontiguousarray(Wk.T).astype(np.float16),
        "wvT": np.ascontiguousarray(Wv.T).astype(np.float16),
        "wuT": np.ascontiguousarray(Wu.T).astype(np.float16),
        "bqr": np.ascontiguousarray(bq.reshape(CT, 128).T).astype(np.float32),
        "bkr": np.ascontiguousarray(bk.reshape(CT, 128).T).astype(np.float32),
        "bvb": np.ascontiguousarray(np.broadcast_to(bv, (128, EMB))).astype(np.float32),
        "bub": np.ascontiguousarray(np.broadcast_to(bu, (128, EMB))).astype(np.float32),
        "sel2": np.kron(np.eye(2), np.ones((1, 64))).astype(np.float16),
    }
    x2T = [x2[b].T.astype(np.float16) for b in range(x1.shape[0])]
    in_maps = []
    n_cores = (x1.shape[0] * x1.shape[1]) // QC
    per_b = x1.shape[1] // QC
    for c in range(n_cores):
        b, q0 = c // per_b, (c % per_b) * QC
        in_maps.append(dict(
            shared,
            x1T=x1[b, q0:q0 + QC, :].T.astype(np.float16),
            x2T=x2T[b],
            maskT=mask[b, q0:q0 + QC, :].T.astype(np.float16),
        ))
    return in_maps


_NC_CACHE = {}


def kernel(x1, x2, mask, Wq, bq, Wk, bk, Wv, bv, Wu, bu):
    cfg = FULL_CFG
    B, TQ, _ = x1.shape
    in_maps = host_prep(np.asarray(x1, np.float32), np.asarray(x2, np.float32),
                        np.asarray(mask), np.asarray(Wq, np.float32),
                        np.asarray(bq, np.float32), np.asarray(Wk, np.float32),
                        np.asarray(bk, np.float32), np.asarray(Wv, np.float32),
                        np.asarray(bv, np.float32), np.asarray(Wu, np.float32),
                        np.asarray(bu, np.float32), cfg)
    key = (cfg["T"], cfg["QC"])
    if key not in _NC_CACHE:
        _NC_CACHE[key] = build(cfg)
    nc = _NC_CACHE[key]
    res = run_bass_kernel_spmd(nc, in_maps, core_ids=list(range(8)),
                               trace=bool(os.environ.get("KERNEL_TRACE")))
    if os.environ.get("KERNEL_TRACE"):
        kernel.last_exec_ns = res.exec_time_ns
        kernel.last_results = res
    out = np.empty((B, TQ, EMB), np.float32)
    per_b = TQ // cfg["QC"]
    for c in range(8):
        b, q0 = c // per_b, (c % per_b) * cfg["QC"]
        out[b, q0:q0 + cfg["QC"], :] = res.results[c]["out"]
    return out


# revision 19
# speedup vs baseline: 1.5889x; 1.0478x over previous
"""Multi-head cross attention on 8 trn2 NeuronCores.

Problem: B=2, T=4096, EMB=512, H=8 heads (head dim 64), fp32 I/O.
  q = x1 @ Wq.T + bq ; k,v from x2 ; S = q k^T / sqrt(512) ;
  softmax over keys with -1e10 masking ; out = (A v) @ Wu.T + bu.

Sharding: core c handles batch b = c//4 and query rows
[1024*(c%4), 1024*(c%4+1)).  Each core computes K,V for its batch in
full (4-way duplication), its own Q chunk, attention, and out-proj.

The kernel is scalar-engine bound: exp() runs only on ACT at 1
elem/cycle/lane (dtype-independent), and the per-core score count is
8 heads x 1024 q x 4096 k = 33.5M elements ~ 255us.  The attention
loop is therefore structured so ACT never stalls:

  - scores for the head pair are computed into TWO per-head PSUM tiles
    sA/sB [128, QC] (2 banks each).  ACT(sA) runs while the next key
    tile's scores overwrite nothing (WAR via pool generation); the
    scores matmuls for key tile kk+1 execute on the PE during ACT(sB)
    of tile kk, so the scalar queue is always fed.
  - AV matmuls for tile kk-1 are emitted after the scores of kk, so
    the PE queue is [scores(kk), AV(kk-1), scores(kk+1), ...] and
    never waits on the exp/mask chain.
  - PSUM budget: sA + sB (4 banks) + avA + avB accumulators (4 banks)
    = all 8 banks; projections run in a serial prologue with their own
    (then freed) PSUM pool.
  - mask tiles stream through a 4-deep SBUF ring, one DMA per
    (pr, kk); the P = exp(S)*mask multiply runs on DVE at 2x fp16
    rate; softmax normalization is deferred via a ones-column in V
    (row 64 of the AV accumulator = denominator), with the reciprocal
    broadcast across partitions on the (otherwise idle) GPSIMD engine.
"""
import math
import os
from contextlib import ExitStack

import numpy as np

import concourse.bass as bass
import concourse.bacc as bacc
import concourse.tile as tile
import concourse.mybir as mybir
from concourse.bass_utils import run_bass_kernel_spmd

F16 = mybir.dt.float16
F32 = mybir.dt.float32
EXP = mybir.ActivationFunctionType.Exp

EMB, H, D, CT = 512, 8, 64, 4  # emb, heads, head dim, emb/128

FULL_CFG = dict(T=4096, QC=1024)  # keys per batch, query rows per core
MINI_CFG = dict(T=512, QC=256)


def attention_body(ctx, tc, io, cfg):
    nc = tc.nc
    T, QC = cfg["T"], cfg["QC"]
    KT = T // 128            # key tiles
    QW = min(512, QC)        # matmul moving width (PSUM bank limit)
    NB = QC // QW            # query blocks
    scale = 1.0 / math.sqrt(EMB)

    pw = ctx.enter_context(tc.tile_pool(name="w", bufs=1))
    pk = ctx.enter_context(tc.tile_pool(name="kt", bufs=1))
    pv = ctx.enter_context(tc.tile_pool(name="v", bufs=1))
    pq = ctx.enter_context(tc.tile_pool(name="qt", bufs=1))
    py = ctx.enter_context(tc.tile_pool(name="y", bufs=1))

    # persistent weights / biases / constants
    wq = [pw.tile([128, EMB], F16, tag=f"wq{i}", name=f"wq{i}") for i in range(CT)]
    wk = [pw.tile([128, EMB], F16, tag=f"wk{i}", name=f"wk{i}") for i in range(CT)]
    wv = [pw.tile([128, EMB], F16, tag=f"wv{i}", name=f"wv{i}") for i in range(CT)]
    wu = [pw.tile([128, EMB], F16, tag=f"wu{i}", name=f"wu{i}") for i in range(CT)]
    bqr = pw.tile([128, CT], F32, tag="bqr", name="bqr")
    bkr = pw.tile([128, CT], F32, tag="bkr", name="bkr")
    bvb = pw.tile([128, EMB], F32, tag="bvb", name="bvb")
    bub = pw.tile([128, EMB], F32, tag="bub", name="bub")

    # preload the exp table early (one-time ~2.7us)
    warm = pw.tile([1, 1], F32, tag="warm", name="warm")
    nc.vector.memset(warm[:], 0.0)
    nc.scalar.activation(warm[:], warm[:], EXP)

    # [1|0] and [0|1] two-column stationaries: both heads' r accumulate in
    # one [2, QC] PSUM region (each head's zero column adds 0 to the other row)
    onz = pw.tile([128, 2, 2], F16, tag="onz", name="onz")
    nc.vector.memset(onz[:], 0.0)
    nc.vector.memset(onz[:, 0, 0:1], 1.0)
    nc.vector.memset(onz[:, 1, 1:2], 1.0)
    # row-select stationaries for the K=2 reciprocal-broadcast matmuls
    sel2 = pw.tile([2, 2 * D], F16, tag="sel2", name="sel2")

    # persistent K^T [emb, T], V [key, head, 64], Q^T [emb, QC]
    kt = [pk.tile([128, T], F16, tag=f"kt{i}", name=f"kt{i}") for i in range(CT)]
    v = pv.tile([128, KT, H, 64], F16, tag="v", name="v")
    qt = [pq.tile([128, QC], F16, tag=f"qt{i}", name=f"qt{i}") for i in range(CT)]
    yts = [py.tile([128, QC], F16, tag=f"yt{e}", name=f"yt{e}") for e in range(CT)]

    # ---- prologue: projections (own PSUM pool, freed before attention) ----
    with tc.tile_pool(name="x", bufs=1) as px, \
         tc.tile_pool(name="pp", bufs=1, space="PSUM") as pp:
        x2t = [px.tile([128, T], F16, tag=f"x2t{i}", name=f"x2t{i}") for i in range(CT)]
        x1t = [px.tile([128, QC], F16, tag=f"x1t{i}", name=f"x1t{i}") for i in range(CT)]
        # DMA issue order = dependency order: the Q-projection inputs first
        # so the PE starts ~5us in, then K inputs, V/out-proj weights last.
        for i in range(CT):
            nc.sync.dma_start(wq[i][:], io["wqT"][bass.ts(i, 128), :])
        nc.sync.dma_start(bqr[:], io["bqr"][:, :])
        for i in range(CT):
            nc.sync.dma_start(x1t[i][:], io["x1T"][bass.ts(i, 128), :])
        for i in range(CT):
            nc.sync.dma_start(wk[i][:], io["wkT"][bass.ts(i, 128), :])
        nc.sync.dma_start(bkr[:], io["bkr"][:, :])
        for hf in range(2):
            for i in range(CT):
                nc.sync.dma_start(x2t[i][:, bass.ts(hf, T // 2)],
                                  io["x2T"][bass.ts(i, 128), bass.ts(hf, T // 2)])
        for i in range(CT):
            nc.sync.dma_start(wv[i][:], io["wvT"][bass.ts(i, 128), :])
        nc.sync.dma_start(bvb[:], io["bvb"][:, :])
        for i in range(CT):
            nc.sync.dma_start(wu[i][:], io["wuT"][bass.ts(i, 128), :])
        nc.sync.dma_start(bub[:], io["bub"][:, :])
        nc.sync.dma_start(sel2[:], io["sel2"][:, :])

        # Q^T[e,q] = sum_c WqT[c,e] * x1T[c,q]  (+ bq per-partition)
        for e in range(CT):
            for t in range(NB):
                ps = pp.tile([128, QW], F32, tag=f"ps{t}", name="ps2")
                for c in range(CT):
                    nc.tensor.matmul(ps[:], wq[c][:, bass.ts(e, 128)],
                                     x1t[c][:, bass.ts(t, QW)],
                                     start=(c == 0), stop=(c == CT - 1))
                nc.vector.tensor_scalar_add(qt[e][:, bass.ts(t, QW)], ps[:],
                                            bqr[:, e:e + 1])
        # K^T[e,t] = sum_c WkT[c,e] * x2T[c,t] (+ bk); V[t,e] interleaved.
        # Stationary (wk) reused across 8 t-chunks per LDWEIGHTS.
        NT8 = min(8, T // 512)
        for e in range(CT):
            for tb in range(T // 512 // NT8):
                pss = [pp.tile([128, 512], F32, tag=f"ps{j}", name=f"ps{j}")
                       for j in range(NT8)]
                for c in range(CT):
                    for j in range(NT8):
                        nc.tensor.matmul(pss[j][:], wk[c][:, bass.ts(e, 128)],
                                         x2t[c][:, bass.ts(tb * NT8 + j, 512)],
                                         start=(c == 0), stop=(c == CT - 1))
                for j in range(NT8):
                    nc.vector.tensor_scalar_add(
                        kt[e][:, bass.ts(tb * NT8 + j, 512)], pss[j][:],
                        bkr[:, e:e + 1])
            if e > 0:
                continue
            # V right after K^T e-tile 0
            for t in range(KT):
                ps = pp.tile([128, EMB], F32, tag="ps0", name="psv")
                for c in range(CT):
                    nc.tensor.matmul(ps[:], x2t[c][:, bass.ts(t, 128)], wv[c][:],
                                     start=(c == 0), stop=(c == CT - 1))
                nc.vector.tensor_add(
                    v[:, t, :, :],
                    ps[:].rearrange("p (h d) -> p h d", h=H),
                    bvb[:].rearrange("p (h d) -> p h d", h=H))

    # ---- attention: ACT-saturated pipeline ----
    # PSUM: sA+sB (4 banks) + av merged col-tiled (2) + r/bc rotation (2) = 8.
    DEF = 3                  # AV/r deferral depth in beats
    TOT = CT * KT            # total beats
    with tc.tile_pool(name="ps_s", bufs=1, space="PSUM") as ps_s, \
         tc.tile_pool(name="ps_av", bufs=1, space="PSUM") as ps_av, \
         tc.tile_pool(name="ps_r", bufs=1, space="PSUM") as ps_r, \
         tc.tile_pool(name="pe", bufs=2) as pe, \
         tc.tile_pool(name="ppt", bufs=DEF + 1) as ppt, \
         tc.tile_pool(name="pm", bufs=4) as pm, \
         tc.tile_pool(name="pn", bufs=2) as pn, \
         tc.tile_pool(name="po", bufs=2) as po:
        avs, rps, ptq = {}, {}, {}

        def mask_dma(b):
            if b < TOT:
                mkt = pm.tile([128, QC], F16, tag="mk", name="mk")
                nc.sync.dma_start(mkt[:], io["maskT"][bass.ts(b % KT, 128), :])
                ptq[("m", b)] = mkt

        def emit_scores(b):
            pr, kk = b // KT, b % KT
            mask_dma(b + 2)
            mkt = ptq.pop(("m", b))
            ss = [ps_s.tile([128, QC], F32, tag=f"s{hh}", name=f"s{hh}")
                  for hh in range(2)]
            # scores: row-packed head pair, separate PSUM tiles
            for cb in range(NB):
                for hh in range(2):
                    nc.tensor.matmul(
                        ss[hh][:, bass.ts(cb, QW)],
                        kt[pr][bass.ds(64 * hh, 64), bass.ts(kk, 128)],
                        qt[pr][bass.ds(64 * hh, 64), bass.ts(cb, QW)],
                        start=True, stop=True,
                        tile_position=(64 * hh, 0))
            pts = []
            for hh in range(2):
                e16 = pe.tile([128, QC], F16, tag=f"e{hh}", name=f"e{hh}")
                nc.scalar.activation(e16[:], ss[hh][:], EXP, scale=scale)
                pt = ppt.tile([128, QC], F16, tag=f"p{hh}", name=f"p{hh}")
                nc.vector.tensor_mul(pt[:], e16[:], mkt[:])
                pts.append(pt)
            ptq[b] = pts

        def emit_avr(b):
            pr, kk = b // KT, b % KT
            pts = ptq.pop(b)
            if kk == 0:
                # both heads' AV accumulators col-tiled into one 2-bank tile:
                # rows 0-63 = head A (array cols 0-63), 64-127 = head B;
                # r accumulator rows 0/1 = heads A/B (onz zero-columns).
                avs[pr] = ps_av.tile([128, QC], F32, tag="av", name="av")
                rps[pr] = ps_r.tile([2, QC], F32, tag="r", name="r")
            av, r_ps = avs[pr], rps[pr]
            for cb in range(NB):
                for hh in range(2):
                    nc.tensor.matmul(
                        av[bass.ds(64 * hh, 64), bass.ts(cb, QW)],
                        v[:, kk, 2 * pr + hh, :],
                        pts[hh][:, bass.ts(cb, QW)],
                        start=(kk == 0), stop=(kk == KT - 1),
                        tile_position=(0, 64 * hh))
            for cb in range(NB):
                for hh in range(2):
                    nc.tensor.matmul(
                        r_ps[:, bass.ts(cb, QW)], onz[:, hh, :],
                        pts[hh][:, bass.ts(cb, QW)],
                        start=(kk == 0 and hh == 0), stop=(kk == KT - 1))
            if kk == KT - 1:
                emit_norm(pr)

        def emit_norm(pr):
            # Y^T_h / r_h via reciprocal + col-tiled K=2 broadcast matmuls
            av, r_ps = avs.pop(pr), rps.pop(pr)
            rr32 = pn.tile([2, QC], F32, tag="rr32", name="rr32")
            nc.vector.reciprocal_approx_fast(rr32[:], r_ps[:])
            rr = pn.tile([2, QC], F16, tag="rr", name="rr")
            with nc.allow_low_precision(reason="fp16 recip copy ok"):
                nc.vector.tensor_copy(rr[:], rr32[:])
            bc = ps_r.tile([128, QC], F32, tag="r", name="bc")
            for hh in range(2):
                for cb in range(NB):
                    nc.tensor.matmul(bc[bass.ds(64 * hh, 64), bass.ts(cb, QW)],
                                     sel2[:, bass.ts(hh, D)],
                                     rr[:, bass.ts(cb, QW)],
                                     start=True, stop=True,
                                     tile_position=(0, 64 * hh))
            bc16 = pn.tile([128, QC], F16, tag="bc16", name="bc16")
            with nc.allow_low_precision(reason="fp16 recip bc ok"):
                nc.vector.tensor_copy(bc16[:], bc[:])
            with nc.allow_low_precision(reason="fp16 y norm ok"):
                nc.vector.tensor_mul(yts[pr][:], av[:], bc16[:])

        mask_dma(0)
        mask_dma(1)
        for b in range(TOT + DEF):
            if b < TOT:
                emit_scores(b)
            if b >= DEF:
                emit_avr(b - DEF)

        # out[q, :] = sum_e Y^T[e, q] * WuT[e, :] + bu
        # (alternating score-bank tags so consecutive q-tiles pipeline)
        for qi in range(QC // 128):
            pso = ps_s.tile([128, max(QC, EMB)], F32, tag=f"s{qi % 2}",
                            name="pso")
            for e in range(CT):
                nc.tensor.matmul(pso[:, 0:EMB], yts[e][:, bass.ts(qi, 128)],
                                 wu[e][:], start=(e == 0), stop=(e == CT - 1))
            osb = po.tile([128, EMB], F32, tag="o", name="osb")
            nc.vector.tensor_add(osb[:], pso[:, 0:EMB], bub[:])
            nc.sync.dma_start(io["out"][bass.ts(qi, 128), :], osb[:])


def build(cfg, num_devices=8):
    T, QC = cfg["T"], cfg["QC"]
    nc = bacc.Bacc("TRN2", target_bir_lowering=False, debug=False,
                   num_devices=num_devices)
    io = {
        "x1T": nc.dram_tensor("x1T", [EMB, QC], F16, kind="ExternalInput").ap(),
        "x2T": nc.dram_tensor("x2T", [EMB, T], F16, kind="ExternalInput").ap(),
        "maskT": nc.dram_tensor("maskT", [T, QC], F16, kind="ExternalInput").ap(),
        "wqT": nc.dram_tensor("wqT", [EMB, EMB], F16, kind="ExternalInput").ap(),
        "wkT": nc.dram_tensor("wkT", [EMB, EMB], F16, kind="ExternalInput").ap(),
        "wvT": nc.dram_tensor("wvT", [EMB, EMB], F16, kind="ExternalInput").ap(),
        "wuT": nc.dram_tensor("wuT", [EMB, EMB], F16, kind="ExternalInput").ap(),
        "bqr": nc.dram_tensor("bqr", [128, CT], F32, kind="ExternalInput").ap(),
        "bkr": nc.dram_tensor("bkr", [128, CT], F32, kind="ExternalInput").ap(),
        "bvb": nc.dram_tensor("bvb", [128, EMB], F32, kind="ExternalInput").ap(),
        "bub": nc.dram_tensor("bub", [128, EMB], F32, kind="ExternalInput").ap(),
        "sel2": nc.dram_tensor("sel2", [2, 2 * D], F16, kind="ExternalInput").ap(),
        "out": nc.dram_tensor("out", [QC, EMB], F32, kind="ExternalOutput").ap(),
    }
    with tile.TileContext(nc) as tc:
        with ExitStack() as ctx:
            attention_body(ctx, tc, io, cfg)
    nc.compile()
    return nc


def host_prep(x1, x2, mask, Wq, bq, Wk, bk, Wv, bv, Wu, bu, cfg):
    """Build the 8 per-core input maps from full inputs."""
    T, QC = cfg["T"], cfg["QC"]
    shared = {
        "wqT": np.ascontiguousarray(Wq.T).astype(np.float16),
        "wkT": np.ascontiguousarray(Wk.T).astype(np.float16),
        "wvT": np.ascontiguousarray(Wv.T).astype(np.float16),
        "wuT": np.ascontiguousarray(Wu.T).astype(np.float16),
        "bqr": np.ascontiguousarray(bq.reshape(CT, 128).T).astype(np.float32),
        "bkr": np.ascontiguousarray(bk.reshape(CT, 128).T).astype(np.float32),
        "bvb": np.ascontiguousarray(np.broadcast_to(bv, (128, EMB))).astype(np.float32),
        "bub": np.ascontiguousarray(np.broadcast_to(bu, (128, EMB))).astype(np.float32),
        "sel2": np.kron(np.eye(2), np.ones((1, 64))).astype(np.float16),
    }
    x2T = [x2[b].T.astype(np.float16) for b in range(x1.shape[0])]
    in_maps = []
    n_cores = (x1.shape[0] * x1.shape[1]) // QC
    per_b = x1.shape[1] // QC
    for c in range(n_cores):
        b, q0 = c // per_b, (c % per_b) * QC
        in_maps.append(dict(
            shared,
            x1T=x1[b, q0:q0 + QC, :].T.astype(np.float16),
            x2T=x2T[b],
            maskT=mask[b, q0:q0 + QC, :].T.astype(np.float16),
        ))
    return in_maps


_NC_CACHE = {}


def kernel(x1, x2, mask, Wq, bq, Wk, bk, Wv, bv, Wu, bu):
    cfg = FULL_CFG
    B, TQ, _ = x1.shape
    in_maps = host_prep(np.asarray(x1, np.float32), np.asarray(x2, np.float32),
                        np.asarray(mask), np.asarray(Wq, np.float32),
                        np.asarray(bq, np.float32), np.asarray(Wk, np.float32),
                        np.asarray(bk, np.float32), np.asarray(Wv, np.float32),
                        np.asarray(bv, np.float32), np.asarray(Wu, np.float32),
                        np.asarray(bu, np.float32), cfg)
    key = (cfg["T"], cfg["QC"])
    if key not in _NC_CACHE:
        _NC_CACHE[key] = build(cfg)
    nc = _NC_CACHE[key]
    res = run_bass_kernel_spmd(nc, in_maps, core_ids=list(range(8)),
                               trace=bool(os.environ.get("KERNEL_TRACE")))
    if os.environ.get("KERNEL_TRACE"):
        kernel.last_exec_ns = res.exec_time_ns
        kernel.last_results = res
    out = np.empty((B, TQ, EMB), np.float32)
    per_b = TQ // cfg["QC"]
    for c in range(8):
        b, q0 = c // per_b, (c % per_b) * cfg["QC"]
        out[b, q0:q0 + cfg["QC"], :] = res.results[c]["out"]
    return out


# revision 45
# speedup vs baseline: 1.9165x; 1.2062x over previous
"""Multi-head cross attention on 8 trn2 NeuronCores.

Problem: B=2, T=4096, EMB=512, H=8 heads (head dim 64), fp32 I/O.
  q = x1 @ Wq.T + bq ; k,v from x2 ; S = q k^T / sqrt(512) ;
  softmax over keys with -1e10 masking ; out = (A v) @ Wu.T + bu.

Sharding: core c handles batch b = c//4 and query rows
[1024*(c%4), 1024*(c%4+1)).  Each core computes K,V for its batch in
full (4-way duplication), its own Q chunk, attention, and out-proj.

The kernel is scalar-engine bound: exp() runs only on ACT at 1
elem/cycle/lane (dtype-independent); the per-core score count is
8 heads x 1024 q x 4096 k = 33.5M elements ~ 285us of ACT.  The
attention runs as a software-pipelined stream of "beats" (one key tile
x one head pair) built so ACT never waits, and so the tensor engine
stays dense enough that its HAM clock-gate holds at 2.4 GHz:

  - per-head score PSUM tiles s0/s1 [128, QC]: the scores matmuls for
    beat b+1 run on the PE while ACT processes beat b (the second
    head's ACT covers the WAR turnaround of the first tile).
  - AV + r matmuls for beat b-3 are emitted after the scores of b, so
    a pr-boundary normalization never blocks the scalar pipeline.
  - both heads' AV accumulate into ONE [128, QC] PSUM tile (head A
    rows 0-63 via array cols 0-63, head B rows 64-127 via
    tile_position (0,64)); softmax denominators accumulate via tiny
    [128,2] ones/zeros stationaries into a single-bank [34, QW] tile
    (query block cb at partition 32*cb, 32-aligned for DVE access).
  - K/V/Q projections for later head pairs stream through a 1-bank
    PSUM tag between beats (one ~2048-cycle chunk per beat), keeping
    the PE warm and shrinking the serial prologue to Q/K for the first
    head pair plus the first few V tiles.
  - normalization: reciprocal on DVE, broadcast across partitions by
    K=2 row-select matmuls (sel2), applied in place to the fp16 copy
    of the accumulators so the PSUM tiles release immediately.
"""
import math
import os
from contextlib import ExitStack

import numpy as np

import concourse.bass as bass
import concourse.bacc as bacc
import concourse.tile as tile
import concourse.mybir as mybir
from concourse.bass_utils import run_bass_kernel_spmd

F16 = mybir.dt.float16
F32 = mybir.dt.float32
EXP = mybir.ActivationFunctionType.Exp

EMB, H, D, CT = 512, 8, 64, 4  # emb, heads, head dim, emb/128

FULL_CFG = dict(T=4096, QC=1024)  # keys per batch, query rows per core
MINI_CFG = dict(T=512, QC=256)


def attention_body(ctx, tc, io, cfg):
    nc = tc.nc
    T, QC = cfg["T"], cfg["QC"]
    KT = T // 128            # key tiles
    QW = min(512, QC)        # matmul moving width (PSUM bank limit)
    NB = QC // QW            # query blocks
    NTB = T // 512           # 512-wide t chunks
    VPRO = min(KT, 10)       # V tiles projected in the serial prologue
    scale = 1.0 / math.sqrt(EMB)

    pw = ctx.enter_context(tc.tile_pool(name="w", bufs=1))
    pk = ctx.enter_context(tc.tile_pool(name="kt", bufs=1))
    pv = ctx.enter_context(tc.tile_pool(name="v", bufs=1))
    pq = ctx.enter_context(tc.tile_pool(name="qt", bufs=1))
    py = ctx.enter_context(tc.tile_pool(name="y", bufs=1))
    px = ctx.enter_context(tc.tile_pool(name="x", bufs=1))

    # persistent weights / biases / constants
    wq = [pw.tile([128, EMB], F16, tag=f"wq{i}", name=f"wq{i}") for i in range(CT)]
    wk = [pw.tile([128, EMB], F16, tag=f"wk{i}", name=f"wk{i}") for i in range(CT)]
    wv = [pw.tile([128, EMB], F16, tag=f"wv{i}", name=f"wv{i}") for i in range(CT)]
    wu = [pw.tile([128, EMB], F16, tag=f"wu{i}", name=f"wu{i}") for i in range(CT)]
    bqr = pw.tile([128, CT], F32, tag="bqr", name="bqr")
    bkr = pw.tile([128, CT], F32, tag="bkr", name="bkr")
    bvb = pw.tile([128, EMB], F32, tag="bvb", name="bvb")
    bub = pw.tile([128, EMB], F32, tag="bub", name="bub")

    # preload the exp table early (one-time ~2.7us)
    warm = pw.tile([1, 1], F32, tag="warm", name="warm")
    nc.vector.memset(warm[:], 0.0)
    nc.scalar.activation(warm[:], warm[:], EXP)

    # [1|0] and [0|1] two-column stationaries: both heads' r accumulate in
    # rows 0/1 of a query-block region (the zero column adds 0 to the
    # other head's row)
    onz = pw.tile([128, 2, 2], F16, tag="onz", name="onz")
    nc.vector.memset(onz[:], 0.0)
    nc.vector.memset(onz[:, 0, 0:1], 1.0)
    nc.vector.memset(onz[:, 1, 1:2], 1.0)
    # row-select stationaries for the K=2 reciprocal-broadcast matmuls
    sel2 = pw.tile([2, 2 * D], F16, tag="sel2", name="sel2")

    # persistent K^T [emb, T], V [key, head, 64], Q^T [emb, QC]
    kt = [pk.tile([128, T], F16, tag=f"kt{i}", name=f"kt{i}") for i in range(CT)]
    v = pv.tile([128, KT, H, 64], F16, tag="v", name="v")
    qt = [pq.tile([128, QC], F16, tag=f"qt{i}", name=f"qt{i}") for i in range(CT)]
    yts = [py.tile([128, QC], F16, tag=f"yt{e}", name=f"yt{e}") for e in range(CT)]
    x2t = [px.tile([128, T], F16, tag=f"x2t{i}", name=f"x2t{i}") for i in range(CT)]
    x1t = [px.tile([128, QC], F16, tag=f"x1t{i}", name=f"x1t{i}") for i in range(CT)]

    # projection chunk emitters, shared by the serial prologue and the
    # per-beat interleave (pool/tag differ)
    def q_chunk(pool, tag, e, t):
        ps = pool.tile([128, QW], F32, tag=tag, name="psq")
        for c in range(CT):
            nc.tensor.matmul(ps[:, 0:QW], wq[c][:, bass.ts(e, 128)],
                             x1t[c][:, bass.ts(t, QW)],
                             start=(c == 0), stop=(c == CT - 1))
        nc.vector.tensor_scalar_add(qt[e][:, bass.ts(t, QW)], ps[:, 0:QW],
                                    bqr[:, e:e + 1])

    def k_chunk(pool, tag, e, tb):
        ps = pool.tile([128, 512], F32, tag=tag, name="psk")
        for c in range(CT):
            nc.tensor.matmul(ps[:], wk[c][:, bass.ts(e, 128)],
                             x2t[c][:, bass.ts(tb, 512)],
                             start=(c == 0), stop=(c == CT - 1))
        nc.vector.tensor_scalar_add(kt[e][:, bass.ts(tb, 512)], ps[:],
                                    bkr[:, e:e + 1])

    def v_chunk(pool, tag, t):
        ps = pool.tile([128, EMB], F32, tag=tag, name="psv")
        for c in range(CT):
            nc.tensor.matmul(ps[:], x2t[c][:, bass.ts(t, 128)], wv[c][:],
                             start=(c == 0), stop=(c == CT - 1))
        nc.vector.tensor_add(
            v[:, t, :, :],
            ps[:].rearrange("p (h d) -> p h d", h=H),
            bvb[:].rearrange("p (h d) -> p h d", h=H))

    # ---- prologue: DMAs + projections needed for the first head pair ----
    with tc.tile_pool(name="pp", bufs=1, space="PSUM") as pp:
        # DMA issue order = dependency order: Q-projection inputs first so
        # the PE starts ~5us in, then K inputs, V/out-proj weights last.
        for i in range(CT):
            nc.sync.dma_start(wq[i][:], io["wqT"][bass.ts(i, 128), :])
        nc.sync.dma_start(bqr[:], io["bqr"][:, :])
        for i in range(CT):
            nc.sync.dma_start(x1t[i][:], io["x1T"][bass.ts(i, 128), :])
        for i in range(CT):
            nc.sync.dma_start(wk[i][:], io["wkT"][bass.ts(i, 128), :])
        nc.sync.dma_start(bkr[:], io["bkr"][:, :])
        for hf in range(2):
            for i in range(CT):
                nc.sync.dma_start(x2t[i][:, bass.ts(hf, T // 2)],
                                  io["x2T"][bass.ts(i, 128), bass.ts(hf, T // 2)])
        for i in range(CT):
            nc.sync.dma_start(wv[i][:], io["wvT"][bass.ts(i, 128), :])
        nc.sync.dma_start(bvb[:], io["bvb"][:, :])
        for i in range(CT):
            nc.sync.dma_start(wu[i][:], io["wuT"][bass.ts(i, 128), :])
        nc.sync.dma_start(bub[:], io["bub"][:, :])
        nc.sync.dma_start(sel2[:], io["sel2"][:, :])

        for t in range(NB):
            q_chunk(pp, f"ps{t % 4}", 0, t)
        for tb in range(NTB):
            k_chunk(pp, f"ps{(tb + NB) % 8}", 0, tb)
        for t in range(VPRO):
            v_chunk(pp, f"ps{t % 8}", t)

    # ---- attention ----
    # PSUM: s0+s1 (4 banks) + av merged (2) + r (1) + proj chunks (1) = 8.
    DEF = 3                  # AV/r deferral depth in beats
    TOT = CT * KT            # total beats
    RP = 2 + 32 * (NB - 1)   # r rows: query block cb at partition 32*cb
    with tc.tile_pool(name="ps_s", bufs=1, space="PSUM") as ps_s, \
         tc.tile_pool(name="ps_av", bufs=1, space="PSUM") as ps_av, \
         tc.tile_pool(name="ps_r", bufs=1, space="PSUM") as ps_r, \
         tc.tile_pool(name="ps_j", bufs=1, space="PSUM") as ps_j, \
         tc.tile_pool(name="pe", bufs=2) as pe, \
         tc.tile_pool(name="ppt", bufs=4) as ppt, \
         tc.tile_pool(name="pm", bufs=4) as pm, \
         tc.tile_pool(name="pn", bufs=2) as pn, \
         tc.tile_pool(name="po", bufs=2) as po:
        avs, rps, ptq = {}, {}, {}

        # remaining projection chunks, one per beat between attention MMs
        work = []
        for t in range(VPRO, KT):
            work.append(lambda t=t: v_chunk(ps_j, "pj", t))
        for e in range(1, CT):
            for t in range(NB):
                work.append(lambda e=e, t=t: q_chunk(ps_j, "pj", e, t))
            for tb in range(NTB):
                work.append(lambda e=e, tb=tb: k_chunk(ps_j, "pj", e, tb))

        def mask_dma(b):
            if b < TOT:
                mkt = pm.tile([128, QC], F16, tag="mk", name="mk")
                nc.sync.dma_start(mkt[:], io["maskT"][bass.ts(b % KT, 128), :])
                ptq[("m", b)] = mkt

        def emit_scores(b):
            pr, kk = b // KT, b % KT
            mask_dma(b + 2)
            mkt = ptq.pop(("m", b))
            ss = [ps_s.tile([128, QC], F32, tag=f"s{hh}", name=f"s{hh}")
                  for hh in range(2)]
            for cb in range(NB):
                for hh in range(2):
                    nc.tensor.matmul(
                        ss[hh][:, bass.ts(cb, QW)],
                        kt[pr][bass.ds(64 * hh, 64), bass.ts(kk, 128)],
                        qt[pr][bass.ds(64 * hh, 64), bass.ts(cb, QW)],
                        start=True, stop=True,
                        tile_position=(64 * hh, 0))
            pts = []
            for hh in range(2):
                e16 = pe.tile([128, QC], F16, tag=f"e{hh}", name=f"e{hh}")
                nc.scalar.activation(e16[:], ss[hh][:], EXP, scale=scale)
                pt = ppt.tile([128, QC], F16, tag=f"p{hh}", name=f"p{hh}")
                nc.vector.tensor_mul(pt[:], e16[:], mkt[:])
                pts.append(pt)
            ptq[b] = pts

        def emit_avr(b):
            pr, kk = b // KT, b % KT
            pts = ptq.pop(b)
            if kk == 0:
                # both heads' AV accumulators col-tiled into one 2-bank
                # tile: rows 0-63 = head A, 64-127 = head B; r rows 0/1 of
                # partition block 32*cb = heads A/B of query block cb.
                avs[pr] = ps_av.tile([128, QC], F32, tag="av", name="av")
                rps[pr] = ps_r.tile([RP, QW], F32, tag="r", name="r")
            av, r_ps = avs[pr], rps[pr]
            for cb in range(NB):
                for hh in range(2):
                    nc.tensor.matmul(
                        av[bass.ds(64 * hh, 64), bass.ts(cb, QW)],
                        v[:, kk, 2 * pr + hh, :],
                        pts[hh][:, bass.ts(cb, QW)],
                        start=(kk == 0), stop=(kk == KT - 1),
                        tile_position=(0, 64 * hh))
            for cb in range(NB):
                for hh in range(2):
                    nc.tensor.matmul(
                        r_ps[bass.ds(32 * cb, 2), :], onz[:, hh, :],
                        pts[hh][:, bass.ts(cb, QW)],
                        start=(kk == 0 and hh == 0), stop=(kk == KT - 1),
                        tile_position=(0, 32 * cb))
            if kk == KT - 1:
                emit_norm(pr)

        def emit_norm(pr):
            # release av (fp16 copy) and r (aligned copies) fast, then
            # divide in place; 1/r broadcast by K=2 select-matmuls.
            av, r_ps = avs.pop(pr), rps.pop(pr)
            with nc.allow_low_precision(reason="fp16 y copy ok"):
                nc.vector.tensor_copy(yts[pr][:], av[:])
            for cb in range(NB):
                rc = pn.tile([2, QW], F32, tag=f"rc{cb}", name="rc")
                nc.vector.tensor_copy(rc[:], r_ps[bass.ds(32 * cb, 2), :])
                rr32 = pn.tile([2, QW], F32, tag=f"rr32{cb}", name="rr32")
                nc.vector.reciprocal_approx_fast(rr32[:], rc[:])
                rr = pn.tile([2, QW], F16, tag=f"rr{cb}", name="rr")
                with nc.allow_low_precision(reason="fp16 recip copy ok"):
                    nc.vector.tensor_copy(rr[:], rr32[:])
                bc = ps_j.tile([128, 512], F32, tag="pj", name="bc")
                for hh in range(2):
                    nc.tensor.matmul(bc[bass.ds(64 * hh, 64), 0:QW],
                                     sel2[:, bass.ts(hh, D)], rr[:],
                                     start=True, stop=True,
                                     tile_position=(0, 64 * hh))
                bc16 = pn.tile([128, QW], F16, tag=f"bc16{cb}", name="bc16")
                with nc.allow_low_precision(reason="fp16 recip bc ok"):
                    nc.vector.tensor_copy(bc16[:], bc[:, 0:QW])
                with nc.allow_low_precision(reason="fp16 y norm ok"):
                    nc.vector.tensor_mul(yts[pr][:, bass.ts(cb, QW)],
                                         yts[pr][:, bass.ts(cb, QW)],
                                         bc16[:])

        mask_dma(0)
        mask_dma(1)
        for b in range(TOT + DEF):
            if b < TOT:
                emit_scores(b)
                if work:
                    work.pop(0)()
            if b >= DEF:
                emit_avr(b - DEF)

        # out[q, :] = sum_e Y^T[e, q] * WuT[e, :] + bu.  Paired q-tiles on
        # alternating score-bank tags; the e<3 partials run while the final
        # pr's normalization is still in flight (only e=3 waits on yts[3]).
        NQ = QC // 128
        for q0 in range(0, NQ, 2):
            qis = [qi for qi in (q0, q0 + 1) if qi < NQ]
            psos = {}
            for qi in qis:
                pso = ps_s.tile([128, max(QC, EMB)], F32, tag=f"s{qi % 2}",
                                name="pso")
                for e in range(CT - 1):
                    nc.tensor.matmul(pso[:, 0:EMB], yts[e][:, bass.ts(qi, 128)],
                                     wu[e][:], start=(e == 0), stop=False)
                psos[qi] = pso
            for qi in qis:
                nc.tensor.matmul(psos[qi][:, 0:EMB],
                                 yts[CT - 1][:, bass.ts(qi, 128)],
                                 wu[CT - 1][:], start=False, stop=True)
            for qi in qis:
                osb = po.tile([128, EMB], F32, tag=f"o{qi % 2}", name="osb")
                nc.vector.tensor_add(osb[:], psos[qi][:, 0:EMB], bub[:])
                nc.sync.dma_start(io["out"][bass.ts(qi, 128), :], osb[:])


def build(cfg, num_devices=8):
    T, QC = cfg["T"], cfg["QC"]
    nc = bacc.Bacc("TRN2", target_bir_lowering=False, debug=False,
                   num_devices=num_devices)
    io = {
        "x1T": nc.dram_tensor("x1T", [EMB, QC], F16, kind="ExternalInput").ap(),
        "x2T": nc.dram_tensor("x2T", [EMB, T], F16, kind="ExternalInput").ap(),
        "maskT": nc.dram_tensor("maskT", [T, QC], F16, kind="ExternalInput").ap(),
        "wqT": nc.dram_tensor("wqT", [EMB, EMB], F16, kind="ExternalInput").ap(),
        "wkT": nc.dram_tensor("wkT", [EMB, EMB], F16, kind="ExternalInput").ap(),
        "wvT": nc.dram_tensor("wvT", [EMB, EMB], F16, kind="ExternalInput").ap(),
        "wuT": nc.dram_tensor("wuT", [EMB, EMB], F16, kind="ExternalInput").ap(),
        "bqr": nc.dram_tensor("bqr", [128, CT], F32, kind="ExternalInput").ap(),
        "bkr": nc.dram_tensor("bkr", [128, CT], F32, kind="ExternalInput").ap(),
        "bvb": nc.dram_tensor("bvb", [128, EMB], F32, kind="ExternalInput").ap(),
        "bub": nc.dram_tensor("bub", [128, EMB], F32, kind="ExternalInput").ap(),
        "sel2": nc.dram_tensor("sel2", [2, 2 * D], F16, kind="ExternalInput").ap(),
        "out": nc.dram_tensor("out", [QC, EMB], F32, kind="ExternalOutput").ap(),
    }
    with tile.TileContext(nc) as tc:
        with ExitStack() as ctx:
            attention_body(ctx, tc, io, cfg)
    nc.compile()
    return nc


def host_prep(x1, x2, mask, Wq, bq, Wk, bk, Wv, bv, Wu, bu, cfg):
    """Build the 8 per-core input maps from full inputs."""
    T, QC = cfg["T"], cfg["QC"]
    shared = {
        "wqT": np.ascontiguousarray(Wq.T).astype(np.float16),
        "wkT": np.ascontiguousarray(Wk.T).astype(np.float16),
        "wvT": np.ascontiguousarray(Wv.T).astype(np.float16),
        "wuT": np.ascontiguousarray(Wu.T).astype(np.float16),
        "bqr": np.ascontiguousarray(bq.reshape(CT, 128).T).astype(np.float32),
        "bkr": np.ascontiguousarray(bk.reshape(CT, 128).T).astype(np.float32),
        "bvb": np.ascontiguousarray(np.broadcast_to(bv, (128, EMB))).astype(np.float32),
        "bub": np.ascontiguousarray(np.broadcast_to(bu, (128, EMB))).astype(np.float32),
        "sel2": np.kron(np.eye(2), np.ones((1, 64))).astype(np.float16),
    }
    x2T = [x2[b].T.astype(np.float16) for b in range(x1.shape[0])]
    in_maps = []
    n_cores = (x1.shape[0] * x1.shape[1]) // QC
    per_b = x1.shape[1] // QC
    for c in range(n_cores):
        b, q0 = c // per_b, (c % per_b) * QC
        in_maps.append(dict(
            shared,
            x1T=x1[b, q0:q0 + QC, :].T.astype(np.float16),
            x2T=x2T[b],
            maskT=mask[b, q0:q0 + QC, :].T.astype(np.float16),
        ))
    return in_maps


_NC_CACHE = {}


def kernel(x1, x2, mask, Wq, bq, Wk, bk, Wv, bv, Wu, bu):
    cfg = FULL_CFG
    B, TQ, _ = x1.shape
    in_maps = host_prep(np.asarray(x1, np.float32), np.asarray(x2, np.float32),
                        np.asarray(mask), np.asarray(Wq, np.float32),
                        np.asarray(bq, np.float32), np.asarray(Wk, np.float32),
                        np.asarray(bk, np.float32), np.asarray(Wv, np.float32),
                        np.asarray(bv, np.float32), np.asarray(Wu, np.float32),
                        np.asarray(bu, np.float32), cfg)
    key = (cfg["T"], cfg["QC"])
    if key not in _NC_CACHE:
        _NC_CACHE[key] = build(cfg)
    nc = _NC_CACHE[key]
    res = run_bass_kernel_spmd(nc, in_maps, core_ids=list(range(8)),
                               trace=bool(os.environ.get("KERNEL_TRACE")))
    if os.environ.get("KERNEL_TRACE"):
        kernel.last_exec_ns = res.exec_time_ns
        kernel.last_results = res
    out = np.empty((B, TQ, EMB), np.float32)
    per_b = TQ // cfg["QC"]
    for c in range(8):
        b, q0 = c // per_b, (c % per_b) * cfg["QC"]
        out[b, q0:q0 + cfg["QC"], :] = res.results[c]["out"]
    return out
